# revision 56
# baseline (speedup 1.0000x reference)
"""AdaptiveMultiLoRALinear Trainium2 kernel (8 NeuronCores, data-parallel).

Math (reference):
    z = x @ W^T + b                                  # [B,S,D]
    m = sum_e scores_e * (x @ A_e @ B_e)             # low-rank adapter mix
      = x @ A_cat @ (scores-scaled B_cat)            # linearity
    gamma = min(0.5*||z|| / (||m|| + eps), 1)        # per-token clamp
    out = z + gamma * m

Distribution: pure data parallel over the B*S = 32768 tokens, 4096 tokens
per core; W / A / B replicated. No collectives.

Per-core device algorithm (tokens on PSUM partitions):
    xT [D, T] bf16 (host-transposed shard)
    z[t,o]   = sum_d xT[d,t] * Wt[d,o]      TensorE, K=D in 8 chunks
    xaT[r,t] = sum_d Ac[d,r] * xT[d,t]      TensorE (gives xa pre-transposed
                                            for the second matmul)
    m[t,o]   = sum_r xaT[r,t] * Bp[r,o]     TensorE
    ||z||^2, ||m||^2 per token via ScalarE activation(Square, accum_out)
    gamma = sqrt(min(0.25 * nz2 / (nm2 + tiny), 1))  (== min(0.5*nz/nm, 1))
    out = z + gamma*m  via ScalarE scale-copy + VectorE add
"""

import os
import numpy as np
import ml_dtypes

N_CORES = 8
BATCH, SEQ, D = 4, 8192, 1024
TOK = BATCH * SEQ              # 32768 tokens total
T = TOK // N_CORES             # 4096 tokens per core
E, RANK = 16, 16
ER = E * RANK                  # 256
P = 128
KO = D // P                    # 8 contraction chunks over D
RC = ER // P                   # 2 contraction chunks over E*r
BLK = 512                      # tokens per x block
NBLK = T // BLK                # 8
SUB = BLK // P                 # 4 token subtiles per block
NFREE = 512                    # matmul moving free-dim (one PSUM bank)
NH = D // NFREE                # 2 column groups for the 1024-wide outputs

C_CLAMP = 0.5
L_START = 0

# fp8 (e4m3, DoubleRow) path for the two low-rank adapter matmuls. The z
# matmul stays bf16 (it dominates the output, fp8 there would blow the
# error budget). A and B' are pre-scaled by SA/SB on the host so their
# ~0.02-sigma entries land in fp8's normal range; the descale folds into
# the Square/Sqrt activation scales for free.
USE_FP8 = os.environ.get("KERNEL_FP8", "1") == "1"
SA = 16.0
SB = 16.0
S8 = SA * SB

_compiled = {}
LAST_EXEC_NS = None


def _maybe_install_ntff_hook():
    """Optional: enable NTFF profiling under axon (used when KERNEL_TRACE=1)."""
    try:
        import sys, types
        import antenv  # noqa: F401
        try:
            import antenv.axon_hooks  # noqa: F401
            return True  # already present
        except ImportError:
            pass
        from trn_agent_boot.trn_boot import _ntff_profile_via_ctypes
        hook = _ntff_profile_via_ctypes("/opt/axon/libaxon_pjrt.so")
        mod = types.ModuleType("antenv.axon_hooks")
        mod.get_axon_ntff_profile_hook = lambda: hook
        mod.set_axon_ntff_profile_hook = lambda h: None
        sys.modules["antenv.axon_hooks"] = mod
        return hook is not None
    except Exception:
        return False


def _build(use_bias: bool, use_fp8: bool):
    import concourse.mybir as mybir
    import concourse.tile as tile
    from concourse import bacc

    bf = mybir.dt.bfloat16
    f8 = mybir.dt.float8e4
    f32 = mybir.dt.float32
    AF = mybir.ActivationFunctionType
    DR = mybir.MatmulPerfMode.DoubleRow
    adt = f8 if use_fp8 else bf      # adapter-path dtype

    nc = bacc.Bacc("TRN2", target_bir_lowering=False, debug=False,
                   num_devices=N_CORES)

    xT = nc.declare_dram_parameter("xT", [D, T], bf, isOutput=False)
    if use_fp8:
        xT8 = nc.declare_dram_parameter("xT8", [D, T], f8, isOutput=False)
    wt = nc.declare_dram_parameter("wt", [D, D], bf, isOutput=False)
    ac = nc.declare_dram_parameter("ac", [D, ER], adt, isOutput=False)
    if use_fp8:
        # host pre-interleaved [p, nh, rc, o'] so each column-half's two
        # K-planes are contiguous in SBUF — required for the dual-XBUS
        # full-rate DoubleRow streaming path
        bp = nc.declare_dram_parameter("bp", [P, NH * RC * NFREE], adt,
                                       isOutput=False)
    else:
        bp = nc.declare_dram_parameter("bp", [ER, D], adt, isOutput=False)
    if use_bias:
        bvec = nc.declare_dram_parameter("bvec", [1, D], f32, isOutput=False)
    out = nc.declare_dram_parameter("out", [T, D], f32, isOutput=True)

    with tile.TileContext(nc) as tc:
        with (
            tc.tile_pool(name="weights", bufs=1) as wpool,
            tc.tile_pool(name="xin", bufs=4) as xpool,
            tc.tile_pool(name="xa", bufs=3) as xapool,
            tc.tile_pool(name="outp", bufs=4) as opool,
            tc.tile_pool(name="sq", bufs=3) as sqpool,
            tc.tile_pool(name="small", bufs=12) as spool,
            # 8 PSUM banks: ph = 4 single-bank [P,512] slots shared by the z
            # column-halves and the transient xaT tiles; pm = 2 double-bank
            # m slots (real double-buffering -> the next m matmul never waits
            # on the gamma chain).
            tc.tile_pool(name="ph", bufs=4, space="PSUM") as ph,
            tc.tile_pool(name="pm", bufs=4, space="PSUM") as pm,
        ):
            # Single SP HW queue for inputs, ordered by first consumer:
            # ac (first xaT mm), x blocks for block 0, wt (first z mm),
            # bp (first m mm).
            xT_r = xT.rearrange("(ko p) t -> p ko t", p=P)
            if use_fp8:
                xT8_r = xT8.rearrange("(ko p) t -> p ko t", p=P)
            def dma_x8b(t0):
                t = xpool.tile([P, KO, BLK], f8, tag="x8b")
                nc.sync.dma_start(out=t[:], in_=xT8_r[:, :, t0:t0 + BLK])
                return t

            def dma_xb(t0):
                t = xpool.tile([P, KO, BLK], bf, tag="xb")
                nc.sync.dma_start(out=t[:], in_=xT_r[:, :, t0:t0 + BLK])
                return t

            ac_sb = wpool.tile([P, KO, ER], adt)
            nc.sync.dma_start(out=ac_sb[:], in_=ac.rearrange("(ko p) r -> p ko r", p=P))
            x8b_t = {}
            if use_fp8:
                # two fp8 x blocks up front: the xaT stage runs one block
                # ahead, covering the PE while wt/xb0 are still in flight
                x8b_t[0] = dma_x8b(0)
                x8b_t[1] = dma_x8b(BLK)
            wt_sb = wpool.tile([P, KO, D], bf)
            wt_r = wt.rearrange("(ko p) o -> p ko o", p=P)
            nc.sync.dma_start(out=wt_sb[:, :, 0:NFREE], in_=wt_r[:, :, 0:NFREE])
            xb_t = {0: dma_xb(0)}
            nc.sync.dma_start(out=wt_sb[:, :, NFREE:D], in_=wt_r[:, :, NFREE:D])
            if use_fp8:
                bp_sb = wpool.tile([P, NH, RC, NFREE], f8)
                nc.sync.dma_start(out=bp_sb[:],
                                  in_=bp.rearrange("p (nh rc o) -> p nh rc o",
                                                   nh=NH, rc=RC))
            else:
                bp_sb = wpool.tile([P, RC, D], bf)
                nc.sync.dma_start(out=bp_sb[:],
                                  in_=bp.rearrange("(rc p) o -> p rc o", p=P))
            if use_bias:
                b_sb = wpool.tile([P, D], f32)
                # broadcast the [1, D] bias over all 128 partitions
                import concourse.bass as bass
                b_bcast = bass.AP(tensor=bvec.ap().tensor, offset=0,
                                  ap=[[0, P], [1, D]])
                nc.sync.dma_start(out=b_sb[:], in_=b_bcast)

            def compute_xa(src):
                # xaT[r, t] for one block (pre-transposed xa)
                xa_sb = xapool.tile([P, RC, BLK], adt, tag="xa_sb")
                for rc in range(RC):
                    xa_ps = ph.tile([P, BLK], f32, tag="ph")
                    if use_fp8:
                        # DoubleRow: virtual K=256 per mm over two ko planes
                        for g in range(KO // 2):
                            nc.tensor.matmul(
                                xa_ps[:],
                                lhsT=ac_sb[:, 2 * g:2 * g + 2, rc * P:(rc + 1) * P],
                                rhs=src[:, 2 * g:2 * g + 2, :],
                                start=(g == 0), stop=(g == KO // 2 - 1),
                                perf_mode=DR,
                            )
                    else:
                        for ko in range(KO):
                            nc.tensor.matmul(
                                xa_ps[:],
                                lhsT=ac_sb[:, ko, rc * P:(rc + 1) * P],
                                rhs=src[:, ko, :],
                                start=(ko == 0), stop=(ko == KO - 1),
                            )
                    nc.vector.tensor_copy(out=xa_sb[:, rc, :], in_=xa_ps[:])
                return xa_sb

            xa_t = {0: compute_xa(x8b_t[0] if use_fp8 else xb_t[0])}

            for blk in range(NBLK):
                t0 = blk * BLK
                xb = xb_t.pop(blk)
                xa_sb = xa_t.pop(blk)

                # prefetch next block's x streams and run its xaT stage a
                # block early (keeps the PE fed while bigger DMAs land)
                if blk + 1 < NBLK:
                    if use_fp8 and blk + 2 < NBLK:
                        x8b_t[blk + 2] = dma_x8b((blk + 2) * BLK)
                    xb_t[blk + 1] = dma_xb((blk + 1) * BLK)
                    if use_fp8:
                        xa_t[blk + 1] = compute_xa(x8b_t.pop(blk + 1))
                    else:
                        xa_t[blk + 1] = compute_xa(xb_t[blk + 1])

                def emit_m(s):
                    # the m halves as adjacent DR mms (full single-bank
                    # PSUM tiles; batched across a subtile pair so the DR
                    # mode-entry cost is shared)
                    ts = slice(s * P, (s + 1) * P)
                    m_h = []
                    for nh in range(NH):
                        ns = slice(nh * NFREE, (nh + 1) * NFREE)
                        m_ps = pm.tile([P, NFREE], f32, tag="m_h")
                        if use_fp8:
                            # one DoubleRow mm covers the whole K=256
                            nc.tensor.matmul(
                                m_ps[:],
                                lhsT=xa_sb[:, :, ts],
                                rhs=bp_sb[:, nh, :, :],
                                start=True, stop=True,
                                perf_mode=DR,
                            )
                        else:
                            for rc in range(RC):
                                nc.tensor.matmul(
                                    m_ps[:],
                                    lhsT=xa_sb[:, rc, ts],
                                    rhs=bp_sb[:, rc, ns],
                                    start=(rc == 0), stop=(rc == RC - 1),
                                )
                        m_h.append(m_ps)
                    return m_h

                def emit_subtile(s):
                    tok = t0 + s * P
                    ts = slice(s * P, (s + 1) * P)

                    # z in two single-bank column halves; each half's norm
                    # contribution + SBUF evacuation runs as soon as the
                    # half is done, so ph slots recycle fast
                    z_sb = opool.tile([P, D], f32, tag="z_sb")
                    nz2h = spool.tile([P, 2], f32, tag="nz2h")
                    for nh in range(NH):
                        ns = slice(nh * NFREE, (nh + 1) * NFREE)
                        z_ps = ph.tile([P, NFREE], f32, tag="ph")
                        for ko in range(KO):
                            nc.tensor.matmul(
                                z_ps[:],
                                lhsT=xb[:, ko, ts],
                                rhs=wt_sb[:, ko, ns],
                                start=(ko == 0), stop=(ko == KO - 1),
                            )
                        if use_bias:
                            nc.vector.tensor_add(out=z_ps[:], in0=z_ps[:],
                                                 in1=b_sb[:, ns])
                        zsq = sqpool.tile([P, NFREE], bf, tag="sq")
                        nc.scalar.activation(out=zsq[:], in_=z_ps[:],
                                             func=AF.Square,
                                             accum_out=nz2h[:, nh:nh + 1])
                        nc.vector.tensor_copy(out=z_sb[:, ns], in_=z_ps[:])

                    m_h = emit_m(s)

                    nm2h = spool.tile([P, 2], f32, tag="nm2h")
                    for nh in range(NH):
                        msq = sqpool.tile([P, NFREE], bf, tag="sqm")
                        nc.scalar.activation(out=msq[:], in_=m_h[nh][:],
                                             func=AF.Square,
                                             scale=(1.0 / S8) if use_fp8 else 1.0,
                                             accum_out=nm2h[:, nh:nh + 1])
                    nm2 = spool.tile([P, 1], f32, tag="nm2")
                    nc.vector.tensor_add(out=nm2[:], in0=nm2h[:, 0:1],
                                         in1=nm2h[:, 1:2])

                    # gamma = min(0.5*sqrt(nz2/nm2), 1)
                    #       = sqrt(0.25 * min(nz2/nm2, 4))
                    nz2 = spool.tile([P, 1], f32, tag="nz2")
                    nc.vector.tensor_add(out=nz2[:], in0=nz2h[:, 0:1],
                                         in1=nz2h[:, 1:2])
                    rm = spool.tile([P, 1], f32, tag="rm")
                    nc.vector.reciprocal(out=rm[:], in_=nm2[:])
                    u = spool.tile([P, 1], f32, tag="u")
                    nc.vector.tensor_scalar(
                        out=u[:], in0=nz2[:], scalar1=rm[:], scalar2=4.0,
                        op0=mybir.AluOpType.mult, op1=mybir.AluOpType.min,
                    )
                    # in fp8 mode gam absorbs the 1/S8 descale of m_ps:
                    # gam' = sqrt(0.25*u)/S8 = sqrt((0.25/S8^2)*u)
                    gscale = C_CLAMP * C_CLAMP
                    if use_fp8:
                        gscale /= S8 * S8
                    gam = spool.tile([P, 1], f32, tag="gam")
                    nc.scalar.activation(out=gam[:], in_=u[:], func=AF.Sqrt,
                                         scale=gscale)

                    # out = gamma*m + z, fused VectorE ops (PSUM + SBUF)
                    o_sb = opool.tile([P, D], f32, tag="o_sb")
                    for nh in range(NH):
                        ns = slice(nh * NFREE, (nh + 1) * NFREE)
                        nc.vector.scalar_tensor_tensor(
                            out=o_sb[:, ns], in0=m_h[nh][:], scalar=gam[:],
                            in1=z_sb[:, ns],
                            op0=mybir.AluOpType.mult, op1=mybir.AluOpType.add,
                        )
                    # stores go out on the GpSimd SWDGE queue so they never
                    # delay the latency-critical xb loads on the SP queue
                    nc.gpsimd.dma_start(out=out[tok:tok + P, :], in_=o_sb[:])

                for s in range(SUB):
                    emit_subtile(s)

    nc.compile()
    return nc


def kernel(x, W, b, A, B_mat, scores, layer_idx):
    global LAST_EXEC_NS
    from concourse.bass_utils import run_bass_kernel_spmd

    x = np.asarray(x)
    W = np.asarray(W, dtype=np.float32)
    b = np.asarray(b, dtype=np.float32)
    A = np.asarray(A, dtype=np.float32)
    B_mat = np.asarray(B_mat, dtype=np.float32)
    scores = np.asarray(scores, dtype=np.float32)
    li = None if layer_idx is None else int(layer_idx)

    bf = ml_dtypes.bfloat16
    f8 = ml_dtypes.float8_e4m3

    # host-side prep: transpose / concat / score-scale, cast to bf16/fp8
    tokens = np.ascontiguousarray(x.reshape(TOK, D).astype(np.float32))
    xT_f32 = np.ascontiguousarray(tokens.T)                        # [D, TOK]
    xT_full = xT_f32.astype(bf)
    wt_h = np.ascontiguousarray(W.T.astype(bf))                    # [D, D]
    ac_f32 = A.transpose(1, 0, 2).reshape(D, ER)
    sc = scores if not (li is not None and li < L_START) else np.zeros_like(scores)
    bp_f32 = (sc[:, None, None] * B_mat).reshape(ER, D)
    if USE_FP8:
        xT8_full = xT_f32.astype(f8)
        ac_h = np.ascontiguousarray((ac_f32 * SA).astype(f8))
        # interleave to [p, nh, rc, o']: planes of each column-half adjacent
        bp_h = np.ascontiguousarray(
            (bp_f32 * SB).reshape(RC, P, NH, NFREE)
            .transpose(1, 2, 0, 3).reshape(P, NH * RC * NFREE).astype(f8))
    else:
        ac_h = np.ascontiguousarray(ac_f32.astype(bf))
        bp_h = np.ascontiguousarray(bp_f32.astype(bf))

    use_bias = bool(np.any(b != 0.0))
    key = ("nc", use_bias, USE_FP8)
    if key not in _compiled:
        _compiled[key] = _build(use_bias, USE_FP8)
    nc = _compiled[key]

    in_maps = []
    for c in range(N_CORES):
        m = {
            "xT": np.ascontiguousarray(xT_full[:, c * T:(c + 1) * T]),
            "wt": wt_h,
            "ac": ac_h,
            "bp": bp_h,
        }
        if USE_FP8:
            m["xT8"] = np.ascontiguousarray(xT8_full[:, c * T:(c + 1) * T])
        if use_bias:
            m["bvec"] = np.ascontiguousarray(b.reshape(1, D))
        in_maps.append(m)

    trace = os.environ.get("KERNEL_TRACE", "0") == "1" and _maybe_install_ntff_hook()
    res = run_bass_kernel_spmd(nc, in_maps, core_ids=list(range(N_CORES)),
                               trace=bool(trace))
    LAST_EXEC_NS = res.exec_time_ns

    out = np.concatenate([res.results[c]["out"] for c in range(N_CORES)], axis=0)
    return np.ascontiguousarray(out.reshape(BATCH, SEQ, D).astype(np.float32))


# revision 57
# speedup vs baseline: 1.3973x; 1.3973x over previous
"""AdaptiveMultiLoRALinear Trainium2 kernel (8 NeuronCores, data-parallel).

Math (reference):
    z = x @ W^T + b                                  # [B,S,D]
    m = sum_e scores_e * (x @ A_e @ B_e)             # low-rank adapter mix
      = x @ A_cat @ (scores-scaled B_cat)            # linearity
    gamma = min(0.5*||z|| / (||m|| + eps), 1)        # per-token clamp
    out = z + gamma * m

Distribution: pure data parallel over the B*S = 32768 tokens, 4096 tokens
per core; W / A / B replicated. No collectives.

Per-core device algorithm (tokens on PSUM partitions):
    xT [D, T] bf16 (host-transposed shard)
    z[t,o]   = sum_d xT[d,t] * Wt[d,o]      TensorE, K=D in 8 chunks
    xaT[r,t] = sum_d Ac[d,r] * xT[d,t]      TensorE (gives xa pre-transposed
                                            for the second matmul)
    m[t,o]   = sum_r xaT[r,t] * Bp[r,o]     TensorE
    ||z||^2, ||m||^2 per token via ScalarE activation(Square, accum_out)
    gamma = sqrt(min(0.25 * nz2 / (nm2 + tiny), 1))  (== min(0.5*nz/nm, 1))
    out = z + gamma*m  via ScalarE scale-copy + VectorE add
"""

import os
import numpy as np
import ml_dtypes

N_CORES = 8
BATCH, SEQ, D = 4, 8192, 1024
TOK = BATCH * SEQ              # 32768 tokens total
T = TOK // N_CORES             # 4096 tokens per core
E, RANK = 16, 16
ER = E * RANK                  # 256
P = 128
KO = D // P                    # 8 contraction chunks over D
RC = ER // P                   # 2 contraction chunks over E*r
BLK = 512                      # tokens per x block
NBLK = T // BLK                # 8
SUB = BLK // P                 # 4 token subtiles per block
NFREE = 512                    # matmul moving free-dim (one PSUM bank)
NH = D // NFREE                # 2 column groups for the 1024-wide outputs

C_CLAMP = 0.5
L_START = 0

# fp8 (e4m3, DoubleRow) path for the two low-rank adapter matmuls. The z
# matmul stays bf16 (it dominates the output, fp8 there would blow the
# error budget). A and B' are pre-scaled by SA/SB on the host so their
# ~0.02-sigma entries land in fp8's normal range; the descale folds into
# the Square/Sqrt activation scales for free.
USE_FP8 = os.environ.get("KERNEL_FP8", "1") == "1"
SA = 16.0
SB = 16.0
S8 = SA * SB

_compiled = {}
LAST_EXEC_NS = None


def _maybe_install_ntff_hook():
    """Optional: enable NTFF profiling under axon (used when KERNEL_TRACE=1)."""
    try:
        import sys, types
        import antenv  # noqa: F401
        try:
            import antenv.axon_hooks  # noqa: F401
            return True  # already present
        except ImportError:
            pass
        from trn_agent_boot.trn_boot import _ntff_profile_via_ctypes
        hook = _ntff_profile_via_ctypes("/opt/axon/libaxon_pjrt.so")
        mod = types.ModuleType("antenv.axon_hooks")
        mod.get_axon_ntff_profile_hook = lambda: hook
        mod.set_axon_ntff_profile_hook = lambda h: None
        sys.modules["antenv.axon_hooks"] = mod
        return hook is not None
    except Exception:
        return False


def _build(use_bias: bool, use_fp8: bool):
    import concourse.mybir as mybir
    import concourse.tile as tile
    from concourse import bacc

    bf = mybir.dt.bfloat16
    f8 = mybir.dt.float8e4
    f32 = mybir.dt.float32
    AF = mybir.ActivationFunctionType
    DR = mybir.MatmulPerfMode.DoubleRow
    adt = f8 if use_fp8 else bf      # adapter-path dtype

    nc = bacc.Bacc("TRN2", target_bir_lowering=False, debug=False,
                   num_devices=N_CORES)

    xT = nc.declare_dram_parameter("xT", [D, T], bf, isOutput=False)
    if use_fp8:
        xT8 = nc.declare_dram_parameter("xT8", [D, T], f8, isOutput=False)
    wt = nc.declare_dram_parameter("wt", [D, D], bf, isOutput=False)
    ac = nc.declare_dram_parameter("ac", [D, ER], adt, isOutput=False)
    if use_fp8:
        # host pre-interleaved [p, nh, rc, o'] so each column-half's two
        # K-planes are contiguous in SBUF — required for the dual-XBUS
        # full-rate DoubleRow streaming path
        bp = nc.declare_dram_parameter("bp", [P, NH * RC * NFREE], adt,
                                       isOutput=False)
    else:
        bp = nc.declare_dram_parameter("bp", [ER, D], adt, isOutput=False)
    if use_bias:
        bvec = nc.declare_dram_parameter("bvec", [1, D], f32, isOutput=False)
    out = nc.declare_dram_parameter("out", [T, D], f32, isOutput=True)

    with tile.TileContext(nc) as tc:
        with (
            tc.tile_pool(name="weights", bufs=1) as wpool,
            tc.tile_pool(name="xin", bufs=4) as xpool,
            tc.tile_pool(name="xa", bufs=3) as xapool,
            tc.tile_pool(name="outp", bufs=4) as opool,
            tc.tile_pool(name="sq", bufs=3) as sqpool,
            tc.tile_pool(name="small", bufs=12) as spool,
            # 8 PSUM banks: ph = 4 single-bank [P,512] slots shared by the z
            # column-halves and the transient xaT tiles; pm = 2 double-bank
            # m slots (real double-buffering -> the next m matmul never waits
            # on the gamma chain).
            tc.tile_pool(name="ph", bufs=4, space="PSUM") as ph,
            tc.tile_pool(name="pm", bufs=4, space="PSUM") as pm,
        ):
            # Single SP HW queue for inputs, ordered by first consumer:
            # ac (first xaT mm), x blocks for block 0, wt (first z mm),
            # bp (first m mm).
            xT_r = xT.rearrange("(ko p) t -> p ko t", p=P)
            if use_fp8:
                xT8_r = xT8.rearrange("(ko p) t -> p ko t", p=P)
            def dma_x8b(t0):
                t = xpool.tile([P, KO, BLK], f8, tag="x8b")
                nc.sync.dma_start(out=t[:], in_=xT8_r[:, :, t0:t0 + BLK])
                return t

            def dma_xb(t0):
                t = xpool.tile([P, KO, BLK], bf, tag="xb")
                nc.sync.dma_start(out=t[:], in_=xT_r[:, :, t0:t0 + BLK])
                return t

            ac_sb = wpool.tile([P, KO, ER], adt)
            nc.sync.dma_start(out=ac_sb[:], in_=ac.rearrange("(ko p) r -> p ko r", p=P))
            x8b_t = {}
            if use_fp8:
                # two fp8 x blocks up front: the xaT stage runs one block
                # ahead, covering the PE while wt/xb0 are still in flight
                x8b_t[0] = dma_x8b(0)
                x8b_t[1] = dma_x8b(BLK)
            wt_sb = wpool.tile([P, KO, D], bf)
            wt_r = wt.rearrange("(ko p) o -> p ko o", p=P)
            nc.sync.dma_start(out=wt_sb[:, :, 0:NFREE], in_=wt_r[:, :, 0:NFREE])
            xb_t = {0: dma_xb(0)}
            nc.sync.dma_start(out=wt_sb[:, :, NFREE:D], in_=wt_r[:, :, NFREE:D])
            if use_fp8:
                bp_sb = wpool.tile([P, NH, RC, NFREE], f8)
                nc.sync.dma_start(out=bp_sb[:],
                                  in_=bp.rearrange("p (nh rc o) -> p nh rc o",
                                                   nh=NH, rc=RC))
            else:
                bp_sb = wpool.tile([P, RC, D], bf)
                nc.sync.dma_start(out=bp_sb[:],
                                  in_=bp.rearrange("(rc p) o -> p rc o", p=P))
            if use_bias:
                b_sb = wpool.tile([P, D], f32)
                # broadcast the [1, D] bias over all 128 partitions
                import concourse.bass as bass
                b_bcast = bass.AP(tensor=bvec.ap().tensor, offset=0,
                                  ap=[[0, P], [1, D]])
                nc.sync.dma_start(out=b_sb[:], in_=b_bcast)

            def compute_xa(src):
                # xaT[r, t] for one block (pre-transposed xa)
                xa_sb = xapool.tile([P, RC, BLK], adt, tag="xa_sb")
                for rc in range(RC):
                    xa_ps = ph.tile([P, BLK], f32, tag="ph")
                    if use_fp8:
                        # DoubleRow: virtual K=256 per mm over two ko planes
                        for g in range(KO // 2):
                            nc.tensor.matmul(
                                xa_ps[:],
                                lhsT=ac_sb[:, 2 * g:2 * g + 2, rc * P:(rc + 1) * P],
                                rhs=src[:, 2 * g:2 * g + 2, :],
                                start=(g == 0), stop=(g == KO // 2 - 1),
                                perf_mode=DR,
                            )
                    else:
                        for ko in range(KO):
                            nc.tensor.matmul(
                                xa_ps[:],
                                lhsT=ac_sb[:, ko, rc * P:(rc + 1) * P],
                                rhs=src[:, ko, :],
                                start=(ko == 0), stop=(ko == KO - 1),
                            )
                    nc.vector.tensor_copy(out=xa_sb[:, rc, :], in_=xa_ps[:])
                return xa_sb

            xa_t = {0: compute_xa(x8b_t[0] if use_fp8 else xb_t[0])}

            for blk in range(NBLK):
                t0 = blk * BLK
                xb = xb_t.pop(blk)
                xa_sb = xa_t.pop(blk)

                # prefetch next block's x streams and run its xaT stage a
                # block early (keeps the PE fed while bigger DMAs land)
                if blk + 1 < NBLK:
                    if use_fp8 and blk + 2 < NBLK:
                        x8b_t[blk + 2] = dma_x8b((blk + 2) * BLK)
                    xb_t[blk + 1] = dma_xb((blk + 1) * BLK)
                    if use_fp8:
                        xa_t[blk + 1] = compute_xa(x8b_t.pop(blk + 1))
                    else:
                        xa_t[blk + 1] = compute_xa(xb_t[blk + 1])

                def emit_m(s):
                    # the m halves as adjacent DR mms (full single-bank
                    # PSUM tiles; batched across a subtile pair so the DR
                    # mode-entry cost is shared)
                    ts = slice(s * P, (s + 1) * P)
                    m_h = []
                    for nh in range(NH):
                        ns = slice(nh * NFREE, (nh + 1) * NFREE)
                        m_ps = pm.tile([P, NFREE], f32, tag="m_h")
                        if use_fp8:
                            # one DoubleRow mm covers the whole K=256
                            nc.tensor.matmul(
                                m_ps[:],
                                lhsT=xa_sb[:, :, ts],
                                rhs=bp_sb[:, nh, :, :],
                                start=True, stop=True,
                                perf_mode=DR,
                            )
                        else:
                            for rc in range(RC):
                                nc.tensor.matmul(
                                    m_ps[:],
                                    lhsT=xa_sb[:, rc, ts],
                                    rhs=bp_sb[:, rc, ns],
                                    start=(rc == 0), stop=(rc == RC - 1),
                                )
                        m_h.append(m_ps)
                    return m_h

                def emit_subtile(s):
                    tok = t0 + s * P
                    ts = slice(s * P, (s + 1) * P)

                    # z in two single-bank column halves; each half's norm
                    # contribution + SBUF evacuation runs as soon as the
                    # half is done, so ph slots recycle fast
                    z_sb = opool.tile([P, D], f32, tag="z_sb")
                    nz2h = spool.tile([P, 2], f32, tag="nz2h")
                    for nh in range(NH):
                        ns = slice(nh * NFREE, (nh + 1) * NFREE)
                        z_ps = ph.tile([P, NFREE], f32, tag="ph")
                        for ko in range(KO):
                            nc.tensor.matmul(
                                z_ps[:],
                                lhsT=xb[:, ko, ts],
                                rhs=wt_sb[:, ko, ns],
                                start=(ko == 0), stop=(ko == KO - 1),
                            )
                        if use_bias:
                            nc.vector.tensor_add(out=z_ps[:], in0=z_ps[:],
                                                 in1=b_sb[:, ns])
                        zsq = sqpool.tile([P, NFREE], bf, tag="sq")
                        nc.scalar.activation(out=zsq[:], in_=z_ps[:],
                                             func=AF.Square,
                                             accum_out=nz2h[:, nh:nh + 1])
                        nc.vector.tensor_copy(out=z_sb[:, ns], in_=z_ps[:])

                    m_h = emit_m(s)

                    nm2h = spool.tile([P, 2], f32, tag="nm2h")
                    for nh in range(NH):
                        msq = sqpool.tile([P, NFREE], bf, tag="sqm")
                        nc.scalar.activation(out=msq[:], in_=m_h[nh][:],
                                             func=AF.Square,
                                             scale=(1.0 / S8) if use_fp8 else 1.0,
                                             accum_out=nm2h[:, nh:nh + 1])
                    nm2 = spool.tile([P, 1], f32, tag="nm2")
                    nc.vector.tensor_add(out=nm2[:], in0=nm2h[:, 0:1],
                                         in1=nm2h[:, 1:2])

                    # gamma = min(0.5*sqrt(nz2/nm2), 1)
                    #       = sqrt(0.25 * min(nz2/nm2, 4))
                    nz2 = spool.tile([P, 1], f32, tag="nz2")
                    nc.vector.tensor_add(out=nz2[:], in0=nz2h[:, 0:1],
                                         in1=nz2h[:, 1:2])
                    rm = spool.tile([P, 1], f32, tag="rm")
                    nc.vector.reciprocal(out=rm[:], in_=nm2[:])
                    u = spool.tile([P, 1], f32, tag="u")
                    nc.vector.tensor_scalar(
                        out=u[:], in0=nz2[:], scalar1=rm[:], scalar2=4.0,
                        op0=mybir.AluOpType.mult, op1=mybir.AluOpType.min,
                    )
                    # in fp8 mode gam absorbs the 1/S8 descale of m_ps:
                    # gam' = sqrt(0.25*u)/S8 = sqrt((0.25/S8^2)*u)
                    gscale = C_CLAMP * C_CLAMP
                    if use_fp8:
                        gscale /= S8 * S8
                    gam = spool.tile([P, 1], f32, tag="gam")
                    nc.scalar.activation(out=gam[:], in_=u[:], func=AF.Sqrt,
                                         scale=gscale)

                    # out = gamma*m + z, fused VectorE ops (PSUM + SBUF)
                    o_sb = opool.tile([P, D], f32, tag="o_sb")
                    for nh in range(NH):
                        ns = slice(nh * NFREE, (nh + 1) * NFREE)
                        nc.vector.scalar_tensor_tensor(
                            out=o_sb[:, ns], in0=m_h[nh][:], scalar=gam[:],
                            in1=z_sb[:, ns],
                            op0=mybir.AluOpType.mult, op1=mybir.AluOpType.add,
                        )
                    # stores go out on the GpSimd SWDGE queue so they never
                    # delay the latency-critical xb loads on the SP queue
                    nc.gpsimd.dma_start(out=out[tok:tok + P, :], in_=o_sb[:])

                for s in range(SUB):
                    emit_subtile(s)

    nc.compile()
    return nc


def kernel(x, W, b, A, B_mat, scores, layer_idx):
    global LAST_EXEC_NS
    from concourse.bass_utils import run_bass_kernel_spmd

    x = np.asarray(x)
    W = np.asarray(W, dtype=np.float32)
    b = np.asarray(b, dtype=np.float32)
    A = np.asarray(A, dtype=np.float32)
    B_mat = np.asarray(B_mat, dtype=np.float32)
    scores = np.asarray(scores, dtype=np.float32)
    li = None if layer_idx is None else int(layer_idx)

    bf = ml_dtypes.bfloat16
    f8 = ml_dtypes.float8_e4m3

    # host-side prep: transpose / concat / score-scale, cast to bf16/fp8
    tokens = np.ascontiguousarray(x.reshape(TOK, D).astype(np.float32))
    xT_f32 = np.ascontiguousarray(tokens.T)                        # [D, TOK]
    xT_full = xT_f32.astype(bf)
    ac_f32 = A.transpose(1, 0, 2).reshape(D, ER)
    sc = scores if not (li is not None and li < L_START) else np.zeros_like(scores)
    bp_f32 = (sc[:, None, None] * B_mat).reshape(ER, D)
    wt_f32 = W.T
    if USE_FP8:
        xT8_full = xT_f32.astype(f8)
        ac8 = (ac_f32 * SA).astype(f8)
        bp8 = (bp_f32 * SB).astype(f8)
        # fold the deterministic LoRA weight-quantization residual into W:
        # exact when gamma==1 (the typical case), and a ~0.04%-of-m
        # perturbation otherwise. Cuts overall rel err ~9.8e-3 -> ~7.1e-3.
        dW = ac_f32 @ bp_f32 - (ac8.astype(np.float32) / SA) @ (
            bp8.astype(np.float32) / SB)
        wt_f32 = wt_f32 + dW
        ac_h = np.ascontiguousarray(ac8)
        # interleave to [p, nh, rc, o']: planes of each column-half adjacent
        bp_h = np.ascontiguousarray(
            bp8.reshape(RC, P, NH, NFREE)
            .transpose(1, 2, 0, 3).reshape(P, NH * RC * NFREE))
    else:
        ac_h = np.ascontiguousarray(ac_f32.astype(bf))
        bp_h = np.ascontiguousarray(bp_f32.astype(bf))
    wt_h = np.ascontiguousarray(wt_f32.astype(bf))                 # [D, D]

    use_bias = bool(np.any(b != 0.0))
    key = ("nc", use_bias, USE_FP8)
    if key not in _compiled:
        _compiled[key] = _build(use_bias, USE_FP8)
    nc = _compiled[key]

    in_maps = []
    for c in range(N_CORES):
        m = {
            "xT": np.ascontiguousarray(xT_full[:, c * T:(c + 1) * T]),
            "wt": wt_h,
            "ac": ac_h,
            "bp": bp_h,
        }
        if USE_FP8:
            m["xT8"] = np.ascontiguousarray(xT8_full[:, c * T:(c + 1) * T])
        if use_bias:
            m["bvec"] = np.ascontiguousarray(b.reshape(1, D))
        in_maps.append(m)

    trace = os.environ.get("KERNEL_TRACE", "0") == "1" and _maybe_install_ntff_hook()
    res = run_bass_kernel_spmd(nc, in_maps, core_ids=list(range(N_CORES)),
                               trace=bool(trace))
    LAST_EXEC_NS = res.exec_time_ns

    out = np.concatenate([res.results[c]["out"] for c in range(N_CORES)], axis=0)
    return np.ascontiguousarray(out.reshape(BATCH, SEQ, D).astype(np.float32))
